# revision 20
# baseline (speedup 1.0000x reference)
"""Trainium2 Bass kernel for nn_GroupPointEncoder.

Reference computation (G=4, B=8, N=2048, F=128):
  std = 2 or 4 per point by label class
  coords = [point_coord, (point_coord + noise*std)[1:]]           # [G,B,N,3]
  normed = (coords - low) / (high - low)
  pe     = interleaved sin/cos embedding, (y,x,z) order            # [G,B,N,384]
  h      = relu(pe @ W1.T + b1)                                    # [G,B,N,512]
  pos    = h @ W2.T + b2                                           # [G,B,N,256]
  query  = label_weight[labels] + pos
  out    = concat([query_pos, query], -1).reshape(G*B, N, 512)

Sharding: data-parallel over the G*B=32 (g,b) pairs, 4 per core, 8 cores.
Each core computes its 4*2048=8192 points' `query` half on device; the
query_pos half is a passthrough assembled on the host.

Host prep: the per-frequency phase argument xq[p,c,i] = s_p*x[c,i] + ph_p
(in Q18 fixed-point turns, ph = pi/2 for the cos rows) is a numpy outer
product, streamed in as [128, 3, T] tiles.

Device pipeline per 512-point tile (3-tile software pipelining keeps the
PE -- the pole -- saturated and HAM-warm at 2.4 GHz):
  DVE : kq = convert(xq -> i32, RTNE);  kfrac = kq & (2^18-1);  q copies
  ACT : pe = Sin(-2pi/2^18 * kfrac + pi);  relu x4 (per-m bias)
  PE  : hp = W1p @ pe            12 bf16 matmuls
        qp = lwb @ onehot + W2 @ h   10 bf16 matmuls
The bitwise AND is an exact mod 2^18 (two's complement), and
sin(pi - x) = sin(x) maps the [0,1) fraction into the Sin spline's
(-pi, pi] valid domain.  No gpsimd (elementwise ops run ~26us there) and
no tensor_tensor (DVE two-tensor ops run ~11us vs ~1-2us tensor_scalar).
Output q returns as bf16 and is upcast on the host.
"""
import sys
import math

sys.path.insert(0, "/opt/trn_rl_repo")

import numpy as np
from contextlib import ExitStack

import concourse.bass as bass
import concourse.tile as tile
from concourse import bacc, library_config, mybir
from concourse.bass_utils import run_bass_kernel_spmd

# problem constants (hardcoded per contract)
G, B, N, F = 4, 8, 2048, 128
NCORES = 8
BPC = B * G // NCORES          # 4 (g,b) pairs per core
NPTS = BPC * N                 # 8192 points per core
T = 512                        # points per tile
NT = NPTS // T                 # 16 tiles
TWO_PI = 2.0 * math.pi
Q = 2 ** 18                    # fixed-point phase scale (1 turn = 2^18)
MASK = Q - 1
F32 = mybir.dt.float32
F32R = mybir.dt.float32r
I32 = mybir.dt.int32
F16 = mybir.dt.float16
BF16 = mybir.dt.bfloat16
NPBF16 = mybir.dt.np(mybir.dt.bfloat16)

_CACHE = {}


def _build_program():
    nc = bacc.Bacc("TRN2", target_bir_lowering=False, debug=False, num_devices=NCORES)

    xq_d = nc.dram_tensor("xq", [NT, 128, 3, T], BF16, kind="ExternalInput").ap()
    pe01_d = nc.dram_tensor("pe01", [2, 128, 3, T], BF16, kind="ExternalInput").ap()
    oh_d = nc.dram_tensor("oh", [10, NT, T], BF16, kind="ExternalInput").ap()
    bv_d = nc.dram_tensor("bv", [128, 1], F32, kind="ExternalInput").ap()
    w1t_d = nc.dram_tensor("w1t", [128, 12, 128], BF16, kind="ExternalInput").ap()
    w2t_d = nc.dram_tensor("w2t", [128, 8, 128], BF16, kind="ExternalInput").ap()
    lwb_d = nc.dram_tensor("lwb", [10, 2, 128], BF16, kind="ExternalInput").ap()
    b1c_d = nc.dram_tensor("b1c", [128, 4], F32, kind="ExternalInput").ap()
    q_d = nc.dram_tensor("q", [256, NPTS], BF16, kind="ExternalOutput").ap()

    with tile.TileContext(nc) as tc, ExitStack() as ctx:
        cpool = ctx.enter_context(tc.tile_pool(name="consts", bufs=1))
        wpool = ctx.enter_context(tc.tile_pool(name="weights", bufs=1))
        xqp = ctx.enter_context(tc.tile_pool(name="xq", bufs=3))
        pep = ctx.enter_context(tc.tile_pool(name="pe", bufs=3))
        hpool = ctx.enter_context(tc.tile_pool(name="h", bufs=3))
        qsp = ctx.enter_context(tc.tile_pool(name="qs", bufs=3))
        psum_h = ctx.enter_context(tc.tile_pool(name="ph", bufs=6, space="PSUM"))
        psum_q = ctx.enter_context(tc.tile_pool(name="pq", bufs=2, space="PSUM"))

        # DMA order = first-need order: w1+pe0 gate the first matmul
        w1all = wpool.tile([128, 12, 128], BF16, name="w1all", tag="w1all")
        # blocks ordered (k,m=0) x3 first so the first matmul group's
        # stationaries arrive before the rest of W1
        nc.sync.dma_start(w1all[:, 0:3, :], w1t_d[:, 0:3, :])
        _w1pos = {}
        _order = [(k, 0) for k in range(3)] + [
            (k, m) for m in (1, 2, 3) for k in range(3)
        ]
        for _j, _km in enumerate(_order):
            _w1pos[_km] = _j
        w1t = [[w1all[:, _w1pos[(k, m)], :] for m in range(4)] for k in range(3)]
        pe0 = pep.tile([128, 3, T], BF16, tag="pe")
        nc.sync.dma_start(pe0[:], pe01_d[0])
        nc.sync.dma_start(w1all[:, 3:12, :], w1t_d[:, 3:12, :])
        b1c = cpool.tile([128, 4], F32)
        nc.sync.dma_start(b1c[:], b1c_d[:])
        ohall = cpool.tile([10, NT, T], BF16, name="ohall", tag="ohall")
        nc.sync.dma_start(ohall[:], oh_d[:])
        w2all = wpool.tile([128, 8, 128], BF16, name="w2all", tag="w2all")
        nc.sync.dma_start(w2all[:], w2t_d[:])
        w2t = [[w2all[:, k * 2 + mp, :] for mp in range(2)] for k in range(4)]
        lwball = wpool.tile([10, 2, 128], BF16, name="lwball", tag="lwball")
        nc.sync.dma_start(lwball[:], lwb_d[:])
        lwb = [lwball[:, mp, :] for mp in range(2)]
        pe1 = pep.tile([128, 3, T], BF16, tag="pe")
        nc.sync.dma_start(pe1[:], pe01_d[1])
        bv = cpool.tile([128, 1], F32)
        nc.sync.dma_start(bv[:], bv_d[:])
        xq2 = xqp.tile([128, 3, T], BF16, tag="xq")
        nc.sync.dma_start(xq2[:], xq_d[2])

        xq_t = [None] * NT
        pe_t = [None] * NT

        Relu = mybir.ActivationFunctionType.Relu
        Sin = mybir.ActivationFunctionType.Sin

        h_t = [None] * NT

        for i in range(NT + 3):
            t, r, u, v = i, i - 1, i - 2, i - 3

            if t == 0:
                pe_t[0] = pe0
            elif t == 1:
                pe_t[1] = pe1
            elif t == 2:
                xq_t[2] = xq2
            elif t < NT:
                xq = xqp.tile([128, 3, T], BF16, tag="xq")
                nc.sync.dma_start(xq[:], xq_d[t])
                xq_t[t] = xq

            # stage 5 runs one tile behind stage 4: its relu inputs finished
            # a full iteration ago, so the k-matmuls never wait
            if 0 <= v < NT:
                h = h_t[v]
                for mp in range(2):
                    qp = psum_q.tile([128, T], F32, tag="qp")
                    nc.tensor.matmul(
                        qp[:], lwb[mp], ohall[:, v, :],
                        start=True, stop=False,
                    )
                    for k in range(4):
                        nc.tensor.matmul(
                            qp[:], w2t[k][mp],
                            h[:, k, :], start=False, stop=(k == 3),
                        )
                    qs = qsp.tile([128, T], BF16, tag="qs")
                    nc.vector.tensor_copy(qs[:], qp[:])
                    nc.sync.dma_start(
                        q_d[mp * 128 : (mp + 1) * 128, v * T : (v + 1) * T], qs[:]
                    )
                h_t[v] = None

            if 0 <= u < NT:
                h = hpool.tile([128, 4, T], BF16, tag="h")
                pe = pe_t[u]
                for m in range(4):
                    hp = psum_h.tile([128, T], F32, tag="hp")
                    for k in range(3):
                        nc.tensor.matmul(
                            hp[:], w1t[k][m], pe[:, k, :],
                            start=(k == 0), stop=(k == 2),
                        )
                    nc.scalar.activation(
                        h[:, m, :], hp[:], Relu, bias=b1c[:, m : m + 1]
                    )
                h_t[u] = h
                pe_t[u] = None

            if 2 <= r < NT:
                peo = pep.tile([128, 3, T], BF16, tag="pe")
                nc.scalar.activation(
                    peo[:], xq_t[r][:], Sin, scale=-TWO_PI, bias=bv[:]
                )
                pe_t[r] = peo
                xq_t[r] = None

    nc.compile()
    return nc


def _host_prep(point_coord, labels, pc_range, noise, label_weight, W1, b1, W2, b2):
    """Build the per-core input maps (host-side sharding + weight prep)."""
    pc32 = np.asarray(point_coord, np.float32)
    lab = np.asarray(labels)
    noi = np.asarray(noise, np.float32)
    rng = np.asarray(pc_range, np.float32)

    small = (lab == 0) | (lab >= 6)
    std = np.where(small, 2.0, 4.0).astype(np.float32)            # [B,N]
    coords = pc32[None] + noi * std[None, :, :, None]             # [G,B,N,3]
    coords[0] = pc32                                              # group 0 originals
    low, high = rng[:3], rng[3:]
    pcs = (coords - low) / (high - low)                           # [G,B,N,3]
    pcs = pcs[..., [1, 0, 2]]   # reference concatenates pe in (y,x,z) order
    onehot = np.eye(10, dtype=np.float32)[np.asarray(lab, np.int64)]  # [B,N,10]

    # feature permutation: kernel row c*128+k -> ref feature c*128+2k (sin),
    # row c*128+64+k -> c*128+2k+1 (cos)
    perm = np.empty(3 * F, np.int64)
    for c in range(3):
        for k in range(64):
            perm[c * 128 + k] = c * 128 + 2 * k
            perm[c * 128 + 64 + k] = c * 128 + 2 * k + 1
    w1p = np.ascontiguousarray(np.asarray(W1, np.float32)[:, perm].T)  # [384,512]
    w2t = np.ascontiguousarray(np.asarray(W2, np.float32).T)           # [512,256]
    lwb = np.asarray(label_weight, np.float32) + np.asarray(b2, np.float32)[None]
    b1c = np.ascontiguousarray(np.asarray(b1, np.float32).reshape(4, 128).T)

    k64 = np.arange(64, dtype=np.float64)
    s64 = 10000.0 ** (-k64 / 64.0)
    s128 = np.concatenate([s64, s64])
    sQ = (s128 * np.float32(Q)).astype(np.float32)        # per-row Q18 scale
    phQ = np.zeros(128, np.float32)
    phQ[64:] = Q // 4                                     # cos rows: +pi/2
    bv = np.full((128, 1), np.pi, np.float32)

    shared = {
        "bv": bv,
        "w1t": np.ascontiguousarray(
            w1p.reshape(3, 128, 4, 128).transpose(1, 0, 2, 3).reshape(128, 12, 128)[
                :, [0, 4, 8, 1, 5, 9, 2, 6, 10, 3, 7, 11], :
            ]
        ).astype(NPBF16),
        "w2t": np.ascontiguousarray(
            w2t.reshape(4, 128, 2, 128).transpose(1, 0, 2, 3).reshape(128, 8, 128)
        ).astype(NPBF16),
        "lwb": np.ascontiguousarray(lwb.reshape(10, 2, 128)).astype(NPBF16),
        "b1c": b1c,
    }

    in_maps = []
    for core in range(NCORES):
        g = core // 2
        b0 = 4 * (core % 2)
        # normalized coords in "turns" per tile: [NT, 3, T]
        pcc = pcs[g, b0 : b0 + 4].reshape(NPTS, 3).T
        pcc = np.ascontiguousarray(pcc.reshape(3, NT, T).transpose(1, 0, 2))
        # phase per frequency row as fraction-of-turn in [0,1), f16:
        # [NT, 128, 3, T].  Sin(-2pi*frac + pi) == sin(2pi*frac).
        ph = pcc[:, None, :, :].astype(np.float64) * s128[
            None, :, None, None
        ] + (phQ / Q)[None, :, None, None]
        frac = np.mod(ph, 1.0)
        xqc = np.ascontiguousarray(frac.astype(NPBF16))
        # first two tiles' sin precomputed (hides the chain latency behind
        # the fixed ~7us engine-boot + weight DMA window)
        pe01 = np.sin(2 * np.pi * frac[:2]).astype(NPBF16)
        ohc = np.ascontiguousarray(
            onehot[b0 : b0 + 4].reshape(NPTS, 10).T.reshape(10, NT, T)
        )
        in_maps.append(
            {"xq": xqc, "pe01": pe01, "oh": ohc.astype(NPBF16), **shared}
        )
    return in_maps


def _get_nc():
    if "nc" not in _CACHE:
        _CACHE["nc"] = _build_program()
    return _CACHE["nc"]


def _run_device(in_maps, trace=False, **kw):
    nc = _get_nc()
    return run_bass_kernel_spmd(nc, in_maps, list(range(NCORES)), trace=trace, **kw)


def kernel(point_coord, labels, pc_range, noise, query_pos, label_weight, W1, b1, W2, b2):
    in_maps = _host_prep(
        point_coord, labels, pc_range, noise, label_weight, W1, b1, W2, b2
    )
    res = _run_device(in_maps)

    qp = np.asarray(query_pos, np.float32)
    out = np.empty((G * B, N, 4 * F), np.float32)
    out[:, :, : 2 * F] = qp.reshape(G * B, N, 2 * F)
    for core in range(NCORES):
        q = res.results[core]["q"].astype(np.float32)    # [256, NPTS]
        q = q.reshape(2 * F, BPC, N).transpose(1, 2, 0)  # [4, N, 256]
        out[4 * core : 4 * core + 4, :, 2 * F :] = q
    return out


# revision 21
# speedup vs baseline: 1.0135x; 1.0135x over previous
"""Trainium2 Bass kernel for nn_GroupPointEncoder.

Reference computation (G=4, B=8, N=2048, F=128):
  std = 2 or 4 per point by label class
  coords = [point_coord, (point_coord + noise*std)[1:]]           # [G,B,N,3]
  normed = (coords - low) / (high - low)
  pe     = interleaved sin/cos embedding, (y,x,z) order            # [G,B,N,384]
  h      = relu(pe @ W1.T + b1)                                    # [G,B,N,512]
  pos    = h @ W2.T + b2                                           # [G,B,N,256]
  query  = label_weight[labels] + pos
  out    = concat([query_pos, query], -1).reshape(G*B, N, 512)

Sharding: data-parallel over the G*B=32 (g,b) pairs, 4 per core, 8 cores.
Each core computes its 4*2048=8192 points' `query` half on device; the
query_pos half is a passthrough assembled on the host.

Host prep: the per-frequency phase argument xq[p,c,i] = s_p*x[c,i] + ph_p
(in Q18 fixed-point turns, ph = pi/2 for the cos rows) is a numpy outer
product, streamed in as [128, 3, T] tiles.

Device pipeline per 512-point tile (3-tile software pipelining keeps the
PE -- the pole -- saturated and HAM-warm at 2.4 GHz):
  DVE : kq = convert(xq -> i32, RTNE);  kfrac = kq & (2^18-1);  q copies
  ACT : pe = Sin(-2pi/2^18 * kfrac + pi);  relu x4 (per-m bias)
  PE  : hp = W1p @ pe            12 bf16 matmuls
        qp = lwb @ onehot + W2 @ h   10 bf16 matmuls
The bitwise AND is an exact mod 2^18 (two's complement), and
sin(pi - x) = sin(x) maps the [0,1) fraction into the Sin spline's
(-pi, pi] valid domain.  No gpsimd (elementwise ops run ~26us there) and
no tensor_tensor (DVE two-tensor ops run ~11us vs ~1-2us tensor_scalar).
Output q returns as bf16 and is upcast on the host.
"""
import sys
import math

sys.path.insert(0, "/opt/trn_rl_repo")

import numpy as np
from contextlib import ExitStack

import concourse.bass as bass
import concourse.tile as tile
from concourse import bacc, library_config, mybir
from concourse.bass_utils import run_bass_kernel_spmd

# problem constants (hardcoded per contract)
G, B, N, F = 4, 8, 2048, 128
NCORES = 8
BPC = B * G // NCORES          # 4 (g,b) pairs per core
NPTS = BPC * N                 # 8192 points per core
T = 512                        # points per tile
NT = NPTS // T                 # 16 tiles
TWO_PI = 2.0 * math.pi
Q = 2 ** 18                    # fixed-point phase scale (1 turn = 2^18)
MASK = Q - 1
F32 = mybir.dt.float32
F32R = mybir.dt.float32r
I32 = mybir.dt.int32
F16 = mybir.dt.float16
BF16 = mybir.dt.bfloat16
NPBF16 = mybir.dt.np(mybir.dt.bfloat16)

_CACHE = {}


def _build_program():
    nc = bacc.Bacc("TRN2", target_bir_lowering=False, debug=False, num_devices=NCORES)

    xq_d = nc.dram_tensor("xq", [NT, 128, 3, T], BF16, kind="ExternalInput").ap()
    pe01_d = nc.dram_tensor("pe01", [2, 128, 3, T], BF16, kind="ExternalInput").ap()
    oh_d = nc.dram_tensor("oh", [10, NT, T], BF16, kind="ExternalInput").ap()
    bv_d = nc.dram_tensor("bv", [128, 1], F32, kind="ExternalInput").ap()
    w1t_d = nc.dram_tensor("w1t", [128, 12, 128], BF16, kind="ExternalInput").ap()
    w2t_d = nc.dram_tensor("w2t", [128, 8, 128], BF16, kind="ExternalInput").ap()
    lwb_d = nc.dram_tensor("lwb", [10, 2, 128], BF16, kind="ExternalInput").ap()
    b1c_d = nc.dram_tensor("b1c", [128, 4], F32, kind="ExternalInput").ap()
    q_d = nc.dram_tensor("q", [256, NPTS], BF16, kind="ExternalOutput").ap()

    with tile.TileContext(nc) as tc, ExitStack() as ctx:
        cpool = ctx.enter_context(tc.tile_pool(name="consts", bufs=1))
        wpool = ctx.enter_context(tc.tile_pool(name="weights", bufs=1))
        xqp = ctx.enter_context(tc.tile_pool(name="xq", bufs=3))
        pep = ctx.enter_context(tc.tile_pool(name="pe", bufs=3))
        hpool = ctx.enter_context(tc.tile_pool(name="h", bufs=2))
        qsp = ctx.enter_context(tc.tile_pool(name="qs", bufs=3))
        psum_h = ctx.enter_context(tc.tile_pool(name="ph", bufs=4, space="PSUM"))
        psum_q = ctx.enter_context(tc.tile_pool(name="pq", bufs=4, space="PSUM"))

        # DMA order = first-need order: w1+pe0 gate the first matmul
        w1all = wpool.tile([128, 12, 128], BF16, name="w1all", tag="w1all")
        # blocks ordered (k,m=0) x3 first so the first matmul group's
        # stationaries arrive before the rest of W1
        nc.sync.dma_start(w1all[:, 0:3, :], w1t_d[:, 0:3, :])
        _w1pos = {}
        _order = [(k, 0) for k in range(3)] + [
            (k, m) for m in (1, 2, 3) for k in range(3)
        ]
        for _j, _km in enumerate(_order):
            _w1pos[_km] = _j
        w1t = [[w1all[:, _w1pos[(k, m)], :] for m in range(4)] for k in range(3)]
        pe0 = pep.tile([128, 3, T], BF16, tag="pe")
        nc.sync.dma_start(pe0[:], pe01_d[0])
        nc.sync.dma_start(w1all[:, 3:12, :], w1t_d[:, 3:12, :])
        b1c = cpool.tile([128, 4], F32)
        nc.sync.dma_start(b1c[:], b1c_d[:])
        ohall = cpool.tile([10, NT, T], BF16, name="ohall", tag="ohall")
        nc.sync.dma_start(ohall[:], oh_d[:])
        w2all = wpool.tile([128, 8, 128], BF16, name="w2all", tag="w2all")
        nc.sync.dma_start(w2all[:], w2t_d[:])
        w2t = [[w2all[:, k * 2 + mp, :] for mp in range(2)] for k in range(4)]
        lwball = wpool.tile([10, 2, 128], BF16, name="lwball", tag="lwball")
        nc.sync.dma_start(lwball[:], lwb_d[:])
        lwb = [lwball[:, mp, :] for mp in range(2)]
        pe1 = pep.tile([128, 3, T], BF16, tag="pe")
        nc.sync.dma_start(pe1[:], pe01_d[1])
        bv = cpool.tile([128, 1], F32)
        nc.sync.dma_start(bv[:], bv_d[:])
        xq2 = xqp.tile([128, 3, T], BF16, tag="xq")
        nc.sync.dma_start(xq2[:], xq_d[2])

        xq_t = [None] * NT
        pe_t = [None] * NT

        Relu = mybir.ActivationFunctionType.Relu
        Sin = mybir.ActivationFunctionType.Sin

        for i in range(NT + 2):
            t, r, u = i, i - 1, i - 2

            if t == 0:
                pe_t[0] = pe0
            elif t == 1:
                pe_t[1] = pe1
            elif t == 2:
                xq_t[2] = xq2
            elif t < NT:
                xq = xqp.tile([128, 3, T], BF16, tag="xq")
                nc.sync.dma_start(xq[:], xq_d[t])
                xq_t[t] = xq

            if u >= 0:
                h = hpool.tile([128, 4, T], BF16, tag="h")
                pe = pe_t[u]
                for m in range(4):
                    hp = psum_h.tile([128, T], F32, tag="hp")
                    for k in range(3):
                        nc.tensor.matmul(
                            hp[:], w1t[k][m], pe[:, k, :],
                            start=(k == 0), stop=(k == 2),
                        )
                    nc.scalar.activation(
                        h[:, m, :], hp[:], Relu, bias=b1c[:, m : m + 1]
                    )


            if 2 <= r < NT:
                peo = pep.tile([128, 3, T], BF16, tag="pe")
                nc.scalar.activation(
                    peo[:], xq_t[r][:], Sin, scale=-TWO_PI, bias=bv[:]
                )
                pe_t[r] = peo
                xq_t[r] = None

            if u >= 0:
                for mp in range(2):
                    qp = psum_q.tile([128, T], F32, tag="qp")
                    nc.tensor.matmul(
                        qp[:], lwb[mp], ohall[:, u, :],
                        start=True, stop=False,
                    )
                    for k in range(4):
                        nc.tensor.matmul(
                            qp[:], w2t[k][mp],
                            h[:, k, :], start=False, stop=(k == 3),
                        )
                    qs = qsp.tile([128, T], BF16, tag="qs")
                    nc.vector.tensor_copy(qs[:], qp[:])
                    nc.sync.dma_start(
                        q_d[mp * 128 : (mp + 1) * 128, u * T : (u + 1) * T], qs[:]
                    )
                pe_t[u] = None

    nc.compile()
    return nc


def _host_prep(point_coord, labels, pc_range, noise, label_weight, W1, b1, W2, b2):
    """Build the per-core input maps (host-side sharding + weight prep)."""
    pc32 = np.asarray(point_coord, np.float32)
    lab = np.asarray(labels)
    noi = np.asarray(noise, np.float32)
    rng = np.asarray(pc_range, np.float32)

    small = (lab == 0) | (lab >= 6)
    std = np.where(small, 2.0, 4.0).astype(np.float32)            # [B,N]
    coords = pc32[None] + noi * std[None, :, :, None]             # [G,B,N,3]
    coords[0] = pc32                                              # group 0 originals
    low, high = rng[:3], rng[3:]
    pcs = (coords - low) / (high - low)                           # [G,B,N,3]
    pcs = pcs[..., [1, 0, 2]]   # reference concatenates pe in (y,x,z) order
    onehot = np.eye(10, dtype=np.float32)[np.asarray(lab, np.int64)]  # [B,N,10]

    # feature permutation: kernel row c*128+k -> ref feature c*128+2k (sin),
    # row c*128+64+k -> c*128+2k+1 (cos)
    perm = np.empty(3 * F, np.int64)
    for c in range(3):
        for k in range(64):
            perm[c * 128 + k] = c * 128 + 2 * k
            perm[c * 128 + 64 + k] = c * 128 + 2 * k + 1
    w1p = np.ascontiguousarray(np.asarray(W1, np.float32)[:, perm].T)  # [384,512]
    w2t = np.ascontiguousarray(np.asarray(W2, np.float32).T)           # [512,256]
    lwb = np.asarray(label_weight, np.float32) + np.asarray(b2, np.float32)[None]
    b1c = np.ascontiguousarray(np.asarray(b1, np.float32).reshape(4, 128).T)

    k64 = np.arange(64, dtype=np.float64)
    s64 = 10000.0 ** (-k64 / 64.0)
    s128 = np.concatenate([s64, s64])
    sQ = (s128 * np.float32(Q)).astype(np.float32)        # per-row Q18 scale
    phQ = np.zeros(128, np.float32)
    phQ[64:] = Q // 4                                     # cos rows: +pi/2
    bv = np.full((128, 1), np.pi, np.float32)

    shared = {
        "bv": bv,
        "w1t": np.ascontiguousarray(
            w1p.reshape(3, 128, 4, 128).transpose(1, 0, 2, 3).reshape(128, 12, 128)[
                :, [0, 4, 8, 1, 5, 9, 2, 6, 10, 3, 7, 11], :
            ]
        ).astype(NPBF16),
        "w2t": np.ascontiguousarray(
            w2t.reshape(4, 128, 2, 128).transpose(1, 0, 2, 3).reshape(128, 8, 128)
        ).astype(NPBF16),
        "lwb": np.ascontiguousarray(lwb.reshape(10, 2, 128)).astype(NPBF16),
        "b1c": b1c,
    }

    in_maps = []
    for core in range(NCORES):
        g = core // 2
        b0 = 4 * (core % 2)
        # normalized coords in "turns" per tile: [NT, 3, T]
        pcc = pcs[g, b0 : b0 + 4].reshape(NPTS, 3).T
        pcc = np.ascontiguousarray(pcc.reshape(3, NT, T).transpose(1, 0, 2))
        # phase per frequency row as fraction-of-turn in [0,1), f16:
        # [NT, 128, 3, T].  Sin(-2pi*frac + pi) == sin(2pi*frac).
        ph = pcc[:, None, :, :].astype(np.float64) * s128[
            None, :, None, None
        ] + (phQ / Q)[None, :, None, None]
        frac = np.mod(ph, 1.0)
        xqc = np.ascontiguousarray(frac.astype(NPBF16))
        # first two tiles' sin precomputed (hides the chain latency behind
        # the fixed ~7us engine-boot + weight DMA window)
        pe01 = np.sin(2 * np.pi * frac[:2]).astype(NPBF16)
        ohc = np.ascontiguousarray(
            onehot[b0 : b0 + 4].reshape(NPTS, 10).T.reshape(10, NT, T)
        )
        in_maps.append(
            {"xq": xqc, "pe01": pe01, "oh": ohc.astype(NPBF16), **shared}
        )
    return in_maps


def _get_nc():
    if "nc" not in _CACHE:
        _CACHE["nc"] = _build_program()
    return _CACHE["nc"]


def _run_device(in_maps, trace=False, **kw):
    nc = _get_nc()
    return run_bass_kernel_spmd(nc, in_maps, list(range(NCORES)), trace=trace, **kw)


def kernel(point_coord, labels, pc_range, noise, query_pos, label_weight, W1, b1, W2, b2):
    in_maps = _host_prep(
        point_coord, labels, pc_range, noise, label_weight, W1, b1, W2, b2
    )
    res = _run_device(in_maps)

    qp = np.asarray(query_pos, np.float32)
    out = np.empty((G * B, N, 4 * F), np.float32)
    out[:, :, : 2 * F] = qp.reshape(G * B, N, 2 * F)
    for core in range(NCORES):
        q = res.results[core]["q"].astype(np.float32)    # [256, NPTS]
        q = q.reshape(2 * F, BPC, N).transpose(1, 2, 0)  # [4, N, 256]
        out[4 * core : 4 * core + 4, :, 2 * F :] = q
    return out
